# revision 23
# baseline (speedup 1.0000x reference)
"""ComplexRNN Trainium2 kernel.

10-layer tanh RNN, B=1024, T=512, D=16, H=30, final FC on last timestep.

Strategy (per core, 8-way batch-parallel, 128 batch rows/core):
  - Hidden-major layout: state h^l lives in SBUF as [30 partitions, 128 batch].
  - Layer wavefront: at step s, layer l computes timestep t = s - l. All
    10 layers advance each step; all dependencies are on step s-1.
  - States packed into 3 "region" windows of 128 partitions (4 slots of 32):
      R0 = [h0 h1 h2 h3], R1 = [h3' h4 h5 h6], R2 = [h6' h7 h8 h9]
    (h3', h6' are duplicates recomputed by narrow matmuls so each layer
    finds its feed + recurrent state inside one 128-partition window).
  - Each region is an INDEPENDENT software-pipelined chain: its own state
    tile, its own psum tile, its own per-step ACTIVATE (tanh). Cross-region
    coupling (dup3: R0->R1, dup6: R1->R2) always reads the PREVIOUS step's
    state, so the three chains skew on ScalarE. ScalarE saturates at
    3 ACTs/step (~258ns each); everything else hides under it.
  - Combined biases ride on a "ones row" (partition 126 of each window),
    self-regenerating through tanh (column 126 maps ones -> 30.0 -> 1.0).
  - inject (x_t @ W_ih0^T) and the dup blocks are 32-col stationaries
    (output partitions 0:32 only); the full-width region matmul is the
    start=True member of each psum accumulation group.
"""

import copy
import numpy as np

import concourse.bass as bass
import concourse.tile as tile
from concourse import mybir
from concourse import bass_utils

N_CORES = 8
B, T, D, H, L = 1024, 512, 16, 30, 10
BC = B // N_CORES          # batch per core = 128
RING = 8                   # resident x blocks
DEPTH = 3                  # state rotation depth
PDEPTH = 2                 # psum rotation depth (PSUM has only 8 banks)

F16 = mybir.dt.float16
F32 = mybir.dt.float32

# wbuf column layout (all matmul weight blocks are 128 cols wide so every
# matmul is full 128-col mode -- no PE tiling-mode switches; and every
# group's start=True member depends on an EARLIER ScalarE slot than its
# stop member, so each region chain's loop contains only one matmul).
# inject uses K=64 blocks: the moving operand is a 64-partition slice of
# the x ring (base partition 0 or 64 -- PE requires 0/32/64); 4 stationary
# variants select t%4 within the slice (other 48 rows are zero).
WC_R0, WC_R1, WC_R2 = 0, 128, 256
WC_INJ = 384               # + 128*q, q=0..3
WC_DUP3 = WC_INJ + 4 * 128
WC_DUP6 = WC_DUP3 + 128
WC_FC = WC_DUP6 + 128
W_COLS = WC_FC + 32


def _split_sync_waits(nc, limit=1):
    """walrus CoreV2/V3 lowering rejects instructions whose sync_info carries
    more than ~1 wait condition. Hoist excess waits onto same-engine NoOps
    inserted immediately before the offending instruction (engines execute
    their stream in order, so the waits still gate it)."""
    for fn in nc.m.functions:
        for blk in fn.blocks:
            newlist = []
            for inst in blk.instructions:
                si = inst.sync_info
                if si is not None and si.on_wait and len(si.on_wait) > limit:
                    waits = list(si.on_wait)
                    extra, keep = waits[:-limit], waits[-limit:]
                    for j, w in enumerate(extra):
                        pre = mybir.InstNoOp(
                            name=f"{inst.name}_w{j}",
                            sync_info=mybir.SyncInfo(on_wait=[w], on_update=[]),
                            bass_nofuse=True,
                            engine=inst.engine,
                        )
                        nc.register_instruction(pre, overwrite=True)
                        newlist.append(pre)
                    inst.sync_info = copy.replace(si, on_wait=keep)
                newlist.append(inst)
            blk.instructions = newlist


def build_kernel(t_steps=T):
    nblk = (t_steps + 7) // 8
    xt_blocks = nblk + 8
    n_steps = t_steps + L - 1  # wavefront steps

    nc = bass.Bass(trn_type="TRN2")
    xt = nc.dram_tensor("xt", [xt_blocks * 128, BC], F16, kind="ExternalInput")
    sinit = nc.dram_tensor("sinit", [128, BC], F16, kind="ExternalInput")
    wbuf = nc.dram_tensor("wbuf", [128, W_COLS], F16, kind="ExternalInput")
    y = nc.dram_tensor("y", [1, BC], F32, kind="ExternalOutput")

    with tile.TileContext(nc) as tc:
        with (
            tc.tile_pool(name="persist", bufs=1) as pp,
            tc.tile_pool(name="psum", bufs=1, space="PSUM") as pq,
        ):
            wt = pp.tile([128, W_COLS], F16, tag="wt", name="wt")
            ring = [pp.tile([128, BC], F16, tag=f"ring{i}", name=f"ring{i}")
                    for i in range(RING)]
            # one tile per region per rotation slot: keeps the three region
            # chains' dependencies independent so they pipeline on ScalarE
            st = [[pp.tile([128, BC], F16, tag=f"st{r}_{i}",
                           name=f"st{r}_{i}") for i in range(DEPTH)]
                  for r in range(3)]
            pt = [[pq.tile([128, BC], F32, tag=f"pt{r}_{i}",
                           name=f"pt{r}_{i}") for i in range(PDEPTH)]
                  for r in range(3)]
            pfc = pq.tile([1, BC], F32, tag="pfc", name="pfc")

            # --- init: DMAs spread across engine queues to parallelize ---
            qs = [nc.sync, nc.scalar, nc.gpsimd]
            wsplit = [0, W_COLS // 3, 2 * W_COLS // 3, W_COLS]
            for i in range(3):
                qs[i].dma_start(out=wt[:, wsplit[i]:wsplit[i + 1]],
                                in_=wbuf[:, wsplit[i]:wsplit[i + 1]])
            for i in range(4):  # blocks 4..7 stream in during steps 0..31
                qs[(i + 1) % 3].dma_start(out=ring[i][:, :],
                                          in_=xt[i * 128:(i + 1) * 128, :])
            for r in range(3):  # only the step -1 rotation slot is read
                qs[(r + 2) % 3].dma_start(out=st[r][(DEPTH - 1)][:, :],
                                          in_=sinit[:, :])

            def emit_inject(s, start):
                """x_t @ W_ih0^T for step s (start of p0's acc group).
                K=64 matmul on a 64-partition ring slice; stationary
                variant q = t%4 masks the other timesteps' rows."""
                blk = (s // 8) % RING
                v = s % 8
                u, q = v // 4, v % 4
                nc.tensor.matmul(pt[0][s % PDEPTH][0:128, 0:BC],
                                 wt[64 * u:64 * u + 64, WC_INJ + 128 * q:
                                    WC_INJ + 128 * q + 128],
                                 ring[blk][64 * u:64 * u + 64, :],
                                 start=start, stop=False,
                                 skip_group_check=True)

            def emit_step(s):
                j = (s - 1) % DEPTH
                k = s % DEPTH
                kp = s % PDEPTH
                r0, r1, r2 = st[0][j], st[1][j], st[2][j]

                if s % 8 == 0:
                    b = s // 8 + 4
                    if b < xt_blocks:
                        nc.sync.dma_start(out=ring[b % RING][:, :],
                                          in_=xt[b * 128:(b + 1) * 128, :])

                inj = s < t_steps
                # R0 chain (loop dep: ACT_R0@s-1 only; inject prefetched)
                nc.tensor.matmul(pt[0][kp][0:128, 0:BC],
                                 wt[:, WC_R0:WC_R0 + 128],
                                 r0[:, :], start=not inj, stop=True,
                                 skip_group_check=True)
                # dup3 (dep: ACT_R0@s-1, early) before R1 (dep: ACT_R1@s-1)
                nc.tensor.matmul(pt[1][kp][0:128, 0:BC],
                                 wt[:, WC_DUP3:WC_DUP3 + 128],
                                 r0[:, :], start=True, stop=False,
                                 skip_group_check=True)
                nc.tensor.matmul(pt[1][kp][0:128, 0:BC],
                                 wt[:, WC_R1:WC_R1 + 128],
                                 r1[:, :], start=False, stop=True,
                                 skip_group_check=True)
                # dup6 (dep: ACT_R1@s-1, early) before R2 (dep: ACT_R2@s-1)
                nc.tensor.matmul(pt[2][kp][0:128, 0:BC],
                                 wt[:, WC_DUP6:WC_DUP6 + 128],
                                 r1[:, :], start=True, stop=False,
                                 skip_group_check=True)
                nc.tensor.matmul(pt[2][kp][0:128, 0:BC],
                                 wt[:, WC_R2:WC_R2 + 128],
                                 r2[:, :], start=False, stop=True,
                                 skip_group_check=True)
                # prefetch next step's inject (no ACT dependency)
                if s + 1 < t_steps:
                    emit_inject(s + 1, start=True)

                for r in range(3):
                    nc.scalar.activation(st[r][k][:, :], pt[r][kp][:, :],
                                         mybir.ActivationFunctionType.Tanh)

                # warmup zeroing: slot h^(s+1) must be 0 before step s+1
                # (cheap memset on the otherwise-idle Vector engine)
                l = s + 1
                if 1 <= l <= 9:
                    reg, p0_, p1_ = [
                        None,
                        (0, 32, 62), (0, 64, 94), (0, 96, 126),
                        (1, 32, 62), (1, 64, 94), (1, 96, 126),
                        (2, 32, 62), (2, 64, 94), (2, 96, 126),
                    ][l]
                    nc.vector.memset(st[reg][k][p0_:p1_, :], 0.0)

            if t_steps > 0:
                emit_inject(0, start=True)
            for s in range(n_steps):
                emit_step(s)

            # ---- FC on h9 of last timestep (slot 3 of R2's last state)
            fin = st[2][(n_steps - 1) % DEPTH][:, :]
            nc.tensor.matmul(pfc[0:1, :], wt[:, WC_FC:WC_FC + 1], fin,
                             start=True, stop=True)
            yout = pp.tile([1, BC], F32, tag="yout", name="yout")
            nc.vector.tensor_copy(yout[0:1, :], pfc[0:1, :])
            nc.sync.dma_start(out=y[:, :], in_=yout[0:1, :])

    _split_sync_waits(nc)
    return nc


def prep_core_inputs(x_core, W_ih0, W_ih, W_hh, b_ih, b_hh, fc_w, fc_b,
                     t_steps=T):
    """Host-side marshaling for one core. x_core: [BC, t_steps, D] fp32."""
    nblk = (t_steps + 7) // 8
    xt_blocks = nblk + 8
    # XT[g*128 + 16*(t%8)+d, b] = x[b, 8g + t%8, d]
    xt = np.zeros((xt_blocks * 128, BC), np.float16)
    xr = np.transpose(x_core, (1, 2, 0))  # [t, d, b]
    tpad = nblk * 8
    if t_steps != tpad:
        xr = np.concatenate([xr, np.zeros((tpad - t_steps, D, BC), xr.dtype)],
                            0)
    xt[:nblk * 128, :] = xr.reshape(nblk * 128, BC)

    wbuf = np.zeros((128, W_COLS), np.float32)

    def put_region(col0, layers):
        # layers: (out_slot, feed_slot_or_None, Wi_or_None, rec_slot, Wh, bias)
        for out_slot, fslot, Wi, rslot, Wh, bias in layers:
            c = col0 + 32 * out_slot
            if Wi is not None:
                wbuf[32 * fslot:32 * fslot + Wi.shape[1], c:c + 30] = Wi.T
            wbuf[32 * rslot:32 * rslot + 30, c:c + 30] = Wh.T
            wbuf[126, c:c + 30] = bias
        wbuf[126, col0 + 126] = 30.0  # ones-row regeneration

    bias = b_ih + b_hh
    put_region(WC_R0, [
        (0, None, None, 0, W_hh[0], bias[0]),
        (1, 0, W_ih[0], 1, W_hh[1], bias[1]),
        (2, 1, W_ih[1], 2, W_hh[2], bias[2]),
        (3, 2, W_ih[2], 3, W_hh[3], bias[3]),
    ])
    put_region(WC_R1, [
        (1, 0, W_ih[3], 1, W_hh[4], bias[4]),
        (2, 1, W_ih[4], 2, W_hh[5], bias[5]),
        (3, 2, W_ih[5], 3, W_hh[6], bias[6]),
    ])
    put_region(WC_R2, [
        (1, 0, W_ih[6], 1, W_hh[7], bias[7]),
        (2, 1, W_ih[7], 2, W_hh[8], bias[8]),
        (3, 2, W_ih[8], 3, W_hh[9], bias[9]),
    ])
    for q in range(4):      # variant q masks to rows of timestep t%4 == q,
        for u in range(2):  # duplicated at both 64-slice bases
            r0_ = 64 * u + 16 * q
            wbuf[r0_:r0_ + 16, WC_INJ + 128 * q:WC_INJ + 128 * q + 30] = \
                W_ih0.T
    # dup3 = layer-3 columns of R0 matrix; dup6 = layer-6 columns of R1
    wbuf[:, WC_DUP3:WC_DUP3 + 30] = wbuf[:, WC_R0 + 96:WC_R0 + 126]
    wbuf[:, WC_DUP6:WC_DUP6 + 30] = wbuf[:, WC_R1 + 96:WC_R1 + 126]
    wbuf[96:126, WC_FC] = fc_w[0]
    wbuf[126, WC_FC] = fc_b[0]

    sinit = np.zeros((128, BC), np.float16)
    sinit[126, :] = 1.0
    return {"xt": xt, "wbuf": wbuf.astype(np.float16), "sinit": sinit}


_CACHE = {}


def run(x, W_ih0, W_ih, W_hh, b_ih, b_hh, fc_w, fc_b, t_steps=T):
    x = np.asarray(x, np.float32)
    args = [np.asarray(a, np.float32) for a in
            (W_ih0, W_ih, W_hh, b_ih, b_hh, fc_w, fc_b)]
    key = t_steps
    if key not in _CACHE:
        _CACHE[key] = build_kernel(t_steps)
    nc = _CACHE[key]
    in_maps = [prep_core_inputs(x[c * BC:(c + 1) * BC], *args, t_steps=t_steps)
               for c in range(N_CORES)]
    res = bass_utils.run_bass_kernel_spmd(nc, in_maps,
                                          core_ids=list(range(N_CORES)))
    out = np.concatenate([res.results[c]["y"].reshape(BC, 1)
                          for c in range(N_CORES)], axis=0)
    return out, res


def kernel(x, W_ih0, W_ih, W_hh, b_ih, b_hh, fc_w, fc_b):
    out, _ = run(x, W_ih0, W_ih, W_hh, b_ih, b_hh, fc_w, fc_b)
    return out


if __name__ == "__main__":
    t_small = 32
    rng = np.random.default_rng(0)
    s = 1.0 / np.sqrt(H)
    x = rng.standard_normal((B, t_small, D)).astype(np.float32)
    W_ih0 = (rng.standard_normal((H, D)) * s).astype(np.float32)
    W_ih = (rng.standard_normal((L - 1, H, H)) * s).astype(np.float32)
    W_hh = (rng.standard_normal((L, H, H)) * s).astype(np.float32)
    b_ih = (rng.standard_normal((L, H)) * s).astype(np.float32)
    b_hh = (rng.standard_normal((L, H)) * s).astype(np.float32)
    fc_w = (rng.standard_normal((1, H)) * s).astype(np.float32)
    fc_b = (rng.standard_normal((1,)) * s).astype(np.float32)

    def ref_np(x):
        out = x
        for l in range(L):
            Wi = W_ih0 if l == 0 else W_ih[l - 1]
            xw = np.einsum("btd,hd->bth", out, Wi) + (b_ih[l] + b_hh[l])
            h = np.zeros((x.shape[0], H), np.float32)
            ys = np.empty((x.shape[0], xw.shape[1], H), np.float32)
            for t in range(xw.shape[1]):
                h = np.tanh(xw[:, t] + h @ W_hh[l].T)
                ys[:, t] = h
            out = ys
        return out[:, -1, :] @ fc_w.T + fc_b

    want = ref_np(x)
    got, _ = run(x, W_ih0, W_ih, W_hh, b_ih, b_hh, fc_w, fc_b, t_steps=t_small)
    err = np.abs(got - want).max() / (np.abs(want).max() + 1e-9)
    print("small-T rel err:", err)
